# revision 29
# baseline (speedup 1.0000x reference)
# kernel.py — self-contained Bass/Trainium2 kernel for nn_DogShitNet69
# 6x (3x3x3 conv + masked-BN + ReLU + mask) on 128^3 voxels, then masked
# 3^3/s2 maxpool. Data-parallel over batch (B=8) across 8 NeuronCores; BN
# stats exchanged per layer via AllGather of per-core partial sums.
#
# Layout (v3): all activations fp16, resident in SBUF. Partition layout of
# layer-i output tiles is CHANNEL-MAJOR: tile t covers yo in [lo,hi), and
# partition p = co*(hi-lo) + (yo-lo). Masked input x*m is resident as one
# fp16 volume (z- and x-padded), so L0 streams HBM exactly once (16MB).
# Conv = per-(dz,dx)-tap fp16 matmuls accumulating in PSUM; the y-conv
# (3 taps + stride) and channel mixing are folded into host-built banded
# lhsT [128, 128] matrices. Activation tiles are z-padded so all tap
# slices are uniform (no edge special-casing).
# For layers 1..3 the y-tile boundary row (the single input row that the
# second output tile needs from the first input tile) is staged into a
# [9*Ci, Zo, Xo] tile (all 9 (dz,dx) shifts as partitions) so its
# contribution is ONE matmul per z-chunk instead of a full duplicated
# 9-tap pass over the first k-tile.
# Masked BN via the p-trick: store p = (conv + CBIG)*m; the next layer
# applies y = relu(s*p + (t - CBIG*s)) which zeroes inactive sites (since
# mean_conv + CBIG > 0).
import numpy as np

import concourse.bass as bass
import concourse.mybir as mybir
from concourse import bacc
from concourse.bass_types import AP
from concourse.tile import TileContext
from concourse.bass_utils import run_bass_kernel_spmd

F32 = mybir.dt.float32
F16 = mybir.dt.float16
I32 = mybir.dt.int32
ADD = mybir.AluOpType.add
MULT = mybir.AluOpType.mult
SUB = mybir.AluOpType.subtract
MAXOP = mybir.AluOpType.max
BYPASS = mybir.AluOpType.bypass
AX = mybir.AxisListType
AF = mybir.ActivationFunctionType

N_CORES = 8
D = 128
CH = [1, 2, 4, 8, 16, 32, 64]
STRIDES = [1, 2, 2, 2, 2, 2]
EPS = 1e-5
CBIG = 2.0

SZ = [D]
for _s in STRIDES:
    SZ.append(SZ[-1] // _s)

SUP0 = 8      # L0 streaming superchunk z rows
RCH = 4       # z rows per L0 conv chunk

BND_LAYERS = (1, 2, 3)   # layers using the staged boundary-row matmul


def out_tiles_for(i):
    # partition tiles of layer i's OUTPUT: yo ranges; partitions are
    # channel-major: p = co*(hi-lo) + (yo-lo)
    per = 128 // CH[i + 1]
    return [(a, min(a + per, SZ[i + 1])) for a in range(0, SZ[i + 1], per)]


def in_tiles_for(i):
    if i == 0:
        return [(0, D)]
    return out_tiles_for(i - 1)


def k_tiles(i, yo_lo, yo_hi):
    s, Yi = STRIDES[i], SZ[i]
    ylo, yhi = max(0, s * yo_lo - 1), min(Yi, s * (yo_hi - 1) + 2)
    out = []
    for it, (tlo, thi) in enumerate(in_tiles_for(i)):
        if max(ylo, tlo) < min(yhi, thi):
            out.append(it)
    return out


def k_tiles_main(i, mci):
    A, B = out_tiles_for(i)[mci]
    kt = k_tiles(i, A, B)
    if i in BND_LAYERS and len(kt) == 2:
        # first k-tile contributes only its last row -> staged boundary path
        kt = kt[1:]
    return kt


def mat_keys_for_layer(i):
    keys = []
    for mci in range(len(out_tiles_for(i))):
        for it in k_tiles_main(i, mci):
            for dz in (0, -1, 1):
                for dx in (-1, 0, 1):
                    keys.append((i, dz, dx, mci, it))
    if i in BND_LAYERS:
        for dx in (-1, 0, 1):
            keys.append((i, 'b', dx, 1, 0))
    return keys


def build_band_matrix(w, i, dz, dx, mci, it):
    s, Ci, Co, Yi = STRIDES[i], CH[i], CH[i + 1], SZ[i]
    A, B = out_tiles_for(i)[mci]
    tlo, thi = in_tiles_for(i)[it]
    K = (thi - tlo) * Ci
    M = (B - A) * Co
    mat = np.zeros((K, M), np.float32)
    for yo in range(A, B):
        for dy in (-1, 0, 1):
            yi = s * yo + dy
            if not (tlo <= yi < thi) or not (0 <= yi < Yi):
                continue
            wv = w[dz + 1, dy + 1, dx + 1]   # [Ci, Co]
            for ci in range(Ci):
                for co in range(Co):
                    mat[ci * (thi - tlo) + (yi - tlo),
                        co * (B - A) + (yo - A)] = wv[ci, co]
    return mat


def build_bnd_matrix(w, i, dx):
    # boundary-row matmul lhsT (one per dx): rows p = dzi*Ci + ci, cols
    # co*(B-A). Contribution of input row yi = s*A - 1 (dy=-1) to output
    # row yo=A of out tile 1.
    Ci, Co = CH[i], CH[i + 1]
    A, B = out_tiles_for(i)[1]
    M = (B - A) * Co
    mat = np.zeros((3 * Ci, M), np.float32)
    for dzi in range(3):
        for ci in range(Ci):
            for co in range(Co):
                mat[dzi * Ci + ci, co * (B - A)] = w[dzi, 0, dx + 1, ci, co]
    return mat


def layer_blob(ws, i):
    """Concatenate layer i's band matrices into [128, ncols] fp16."""
    w = np.asarray(ws[i], np.float32)
    mats = []
    for key in mat_keys_for_layer(i):
        if key[1] == 'b':
            mats.append(build_bnd_matrix(w, i, key[2]))
        else:
            _, dz, dx, mci, it = key
            mats.append(build_band_matrix(w, i, dz, dx, mci, it))
    cols = []
    for m in mats:
        K, M = m.shape
        pad = np.zeros((128, M), np.float32)
        pad[:K] = m
        cols.append(pad)
    return np.concatenate(cols, axis=1).astype(np.float16)


def build_consts16():
    """fp16 constants: mask pair-pooling matrices (0/1 -> exact)."""
    c = {}
    for l in range(1, 6):
        Yin, Yout, Co = SZ[l], SZ[l + 1], CH[l + 1]
        u = np.zeros((Yin, Yout), np.float32)
        for yo in range(Yout):
            u[2 * yo, yo] = 1.0
            u[2 * yo + 1, yo] = 1.0
        c[f"pair_u{l}"] = u.astype(np.float16)
        for t, (lo, hi) in enumerate(out_tiles_for(l)):
            d = np.zeros((Yin, (hi - lo) * Co), np.float32)
            for yo in range(lo, hi):
                for co in range(Co):
                    d[2 * yo, co * (hi - lo) + (yo - lo)] = 1.0
                    d[2 * yo + 1, co * (hi - lo) + (yo - lo)] = 1.0
            c[f"pair_d{l}_{t}"] = d.astype(np.float16)
    return c


def build_consts32():
    c = {}
    for i in range(6):
        Co = CH[i + 1]
        per = 128 // Co
        ch = np.zeros((128, Co), np.float32)
        bc = np.zeros((Co, 128), np.float32)
        for p in range(128):
            ch[p, p // per] = 1.0
            bc[p // per, p] = 1.0
        c[f"chmap{i}"] = ch
        c[f"bcast{i}"] = bc
    c["ones_1_64"] = np.ones((1, 64), np.float32)
    c["ones_128_1"] = np.ones((128, 1), np.float32)
    c["ones_128_64"] = np.ones((128, 64), np.float32)
    return c


def _meta(dct):
    meta, off = {}, 0
    for k in sorted(dct.keys(), key=str):
        sh = dct[k].shape
        meta[k] = (off, sh)
        off += int(np.prod(sh))
    return meta, off


def _blob_ap(dram, meta, key):
    off, sh = meta[key]
    if len(sh) == 1:
        return AP(dram, off, [[1, sh[0]]])
    return AP(dram, off, [[sh[1], sh[0]], [1, sh[1]]])


_CACHE = {}


def build_program(unroll=1):
    wmeta = {}
    off = 0
    for i in range(6):
        ncols = 128 * len(mat_keys_for_layer(i))
        wmeta[i] = (off, ncols)
        off += 128 * ncols
    wlen = off
    c16_meta, c16len = _meta(build_consts16())
    c32_meta, c32len = _meta(build_consts32())

    nc = bacc.Bacc("TRN2", target_bir_lowering=False, debug=False, num_devices=N_CORES)
    x_d = nc.dram_tensor("x", [D, D, D], F32, kind="ExternalInput")
    m_d = nc.dram_tensor("m", [D, D, D], I32, kind="ExternalInput")
    gb_d = {}
    for i in range(6):
        gb_d[f"g{i}"] = nc.dram_tensor(f"g{i}", [CH[i + 1]], F32, kind="ExternalInput")
        gb_d[f"b{i}"] = nc.dram_tensor(f"b{i}", [CH[i + 1]], F32, kind="ExternalInput")
    wblob_d = nc.dram_tensor("wblob", [wlen], F16, kind="ExternalInput")
    c16_d = nc.dram_tensor("c16", [c16len], F16, kind="ExternalInput")
    c32_d = nc.dram_tensor("c32", [c32len], F32, kind="ExternalInput")
    out_d = nc.dram_tensor("out", [64, 8], F32, kind="ExternalOutput")
    cc_in = [nc.dram_tensor(f"ccin{i}", [3 * CH[i + 1]], F32) for i in range(6)]
    cc_out = [nc.dram_tensor(f"ccout{i}", [N_CORES * 3 * CH[i + 1]], F32,
                             addr_space="Shared")
              for i in range(6)]

    with TileContext(nc) as tc:
        for _u in range(unroll):
            _body(nc, tc, x_d, m_d, gb_d, wblob_d, wmeta, c16_d, c16_meta,
                  c32_d, c32_meta, cc_in, cc_out, out_d)
    nc.compile()
    return nc, wmeta, c16_meta, c32_meta


def _body(nc, tc, x_d, m_d, gb_d, wblob_d, wmeta, c16_d, c16_meta,
          c32_d, c32_meta, cc_in, cc_out, out_d):
    from contextlib import ExitStack
    es = ExitStack()
    persist = es.enter_context(tc.tile_pool(name="persist", bufs=1))
    stream = es.enter_context(tc.tile_pool(name="stream", bufs=2))
    psum = es.enter_context(tc.tile_pool(name="psum", bufs=6, space="PSUM"))
    psmall = es.enter_context(tc.tile_pool(name="psmall", bufs=2, space="PSUM"))
    scratch = es.enter_context(tc.tile_pool(name="scratch", bufs=1))
    wpool = es.enter_context(tc.tile_pool(name="wts", bufs=2))
    RG = [list(range(N_CORES))]

    # persistent activation tiles for layers 0..5 (z- and x-padded by 2)
    p_tiles = {}
    for i in range(6):
        Co = CH[i + 1]
        Zo = Xo = SZ[i + 1]
        p_tiles[i] = [persist.tile([(hi - lo) * Co, Zo + 2, Xo + 2], F16,
                                   name=f"p{i}_{t}", tag=f"p{i}_{t}")
                      for t, (lo, hi) in enumerate(out_tiles_for(i))]
        for pt in p_tiles[i]:
            # zero x-pad columns and z-pad rows; interior fully rewritten
            nc.vector.memset(pt[:, :, 0:1], 0.0)
            nc.vector.memset(pt[:, :, Xo + 1:Xo + 2], 0.0)
            nc.vector.memset(pt[:, 0:1, 1:Xo + 1], 0.0)
            nc.vector.memset(pt[:, Zo + 1:Zo + 2, 1:Xo + 1], 0.0)
    # resident masked input x*m fp16, z/x-padded
    xfull = persist.tile([128, D + 2, D + 2], F16, name="xfull", tag="xfull")
    nc.vector.memset(xfull[:, :, 0:1], 0.0)
    nc.vector.memset(xfull[:, :, D + 1:D + 2], 0.0)
    nc.vector.memset(xfull[:, 0:1, 1:D + 1], 0.0)
    nc.vector.memset(xfull[:, D + 1:D + 2, 1:D + 1], 0.0)
    # hp: zx-pooled m0 at y=128; +1 x-pad keeps views unmergeable
    hp = persist.tile([128, D // 2, D // 2 + 1], F16, name="hp", tag="hp")
    m_undup, m_dup = {}, {}
    for l in range(1, 6):
        R, Co = SZ[l + 1], CH[l + 1]
        dt_u = F32 if l == 5 else F16
        m_undup[l] = persist.tile([R, R, R], dt_u, name=f"mu{l}", tag=f"mu{l}")
        m_dup[l] = [persist.tile([(hi - lo) * Co, R, R], F16,
                                 name=f"md{l}_{t}", tag=f"md{l}_{t}")
                    for t, (lo, hi) in enumerate(out_tiles_for(l))]
    cnt_pc = persist.tile([64, 6], F32, name="cnt_pc", tag="cnt_pc")
    stats_sum = persist.tile([128, 64], F32, name="st_sum", tag="st_sum")
    stats_sq = persist.tile([128, 64], F32, name="st_sq", tag="st_sq")
    stats_red = persist.tile([128, 2], F32, name="st_red", tag="st_red")
    stvec = {i: persist.tile([128, 2], F32, name=f"stv{i}", tag=f"stv{i}") for i in range(6)}
    cnt_cols = persist.tile([128, 16], F32, name="cnt_cols", tag="cnt_cols")
    cmaps = {}

    def cm(key, f16=False):
        if key not in cmaps:
            d, meta, dt = (c16_d, c16_meta, F16) if f16 else (c32_d, c32_meta, F32)
            off, sh = meta[key]
            t = persist.tile(list(sh), dt, name=f"c_{key}", tag=f"c_{key}")
            nc.scalar.dma_start(out=t[...], in_=_blob_ap(d, meta, key))
            cmaps[key] = t
        return cmaps[key]

    def mm(out_ap, lhs_ap, rhs_ap, start, stop):
        nc.tensor.matmul(out_ap, lhs_ap, rhs_ap, start=start, stop=stop)

    def ps_small():
        return psmall.tile([128, 512], F32, name="pssmall", tag="small")

    # weight blobs: one DMA per layer, fp16 [128, ncols]
    def load_weights(i):
        off, ncols = wmeta[i]
        wt = wpool.tile([128, ncols], F16, name=f"w{i}blob", tag="wblob")
        nc.scalar.dma_start(out=wt[...], in_=AP(wblob_d, off, [[ncols, 128], [1, ncols]]))
        keys = mat_keys_for_layer(i)
        return {k: wt[:, j * 128:(j + 1) * 128] for j, k in enumerate(keys)}

    TAPS = [(dz, dx) for dz in (0, -1, 1) for dx in (-1, 0, 1)]  # dz=0 first

    # =======================================================================
    # mask pyramid (pair matmuls fp16; sums of 0/1 exact). Level 1 is built
    # incrementally inside the L0 loop; levels 2..5 overlap AG(0).
    # =======================================================================
    def mask_level_chunk(l, src_ap3, nx, zr, zr1, eng=None):
        eng = eng or nc.vector
        Co, Rl = CH[l + 1], SZ[l + 1]
        cw = (zr1 - zr) * nx
        c0 = zr * nx
        pp = ps_small()
        mm(pp[0:Rl, 0:cw], cm(f"pair_u{l}", f16=True)[...], src_ap3[:, zr:zr1, :],
           start=True, stop=True)
        eng.tensor_scalar_min(
            out=m_undup[l][...].rearrange("p a b -> p (a b)")[:, c0:c0 + cw],
            in0=pp[0:Rl, 0:cw], scalar1=1.0)
        for t in range(len(out_tiles_for(l))):
            lo, hi = out_tiles_for(l)[t]
            M = (hi - lo) * Co
            pp2 = ps_small()
            mm(pp2[0:M, 0:cw], cm(f"pair_d{l}_{t}", f16=True)[...],
               src_ap3[:, zr:zr1, :], start=True, stop=True)
            eng.tensor_scalar_min(
                out=m_dup[l][t][...].rearrange("p a b -> p (a b)")[:, c0:c0 + cw],
                in0=pp2[0:M, 0:cw], scalar1=1.0)

    def mask_level_cnt(l):
        # per-channel-replicated count: ones-matmul [R,C] so the AG payload
        # carries cnt per channel (no post-AG broadcast needed)
        C = CH[l + 1]
        r1 = scratch.tile([SZ[l + 1], 1], F32, name="cntred", tag="cntred")
        nc.vector.tensor_reduce(out=r1[...], in_=m_undup[l][...], axis=AX.XY, op=ADD)
        po = ps_small()
        mm(po[0:C, 0:1], cm("ones_128_64")[0:SZ[l + 1], 0:C], r1[...],
           start=True, stop=True)
        nc.vector.tensor_copy(out=cnt_pc[0:C, l:l + 1], in_=po[0:C, 0:1])

    def build_mask_level(l, src_ap3, nz, nx):
        rows_per = max(1, 512 // nx)
        for zr in range(0, nz, rows_per):
            mask_level_chunk(l, src_ap3, nx, zr, min(nz, zr + rows_per))
        mask_level_cnt(l)

    def pyrbuf():
        # shared 3D scratch for the pyramid hx (zx-pool) intermediates
        return scratch.tile([64, 64, 33], F16, name="pyrbuf", tag="pyrbuf")

    def build_pyramid_upper():
        mask_level_cnt(1)
        for l in range(2, 6):
            Rp = SZ[l]
            prev = m_undup[l - 1]
            hx2 = pyrbuf()[0:Rp, 0:Rp, 0:Rp // 2 + 1]  # [Rp,Rp,Rp/2+1] view
            nc.vector.tensor_tensor(out=hx2[:, :, 0:Rp // 2], in0=prev[:, :, 0:Rp:2],
                                    in1=prev[:, :, 1:Rp:2], op=MAXOP)
            hz = scratch.tile([64, 32, 33], F16, name="pyrz", tag="pyrz")[
                0:Rp, 0:Rp // 2, 0:Rp // 2 + 1]
            nc.vector.tensor_tensor(out=hz[:, :, 0:Rp // 2], in0=hx2[:, 0:Rp:2, 0:Rp // 2],
                                    in1=hx2[:, 1:Rp:2, 0:Rp // 2], op=MAXOP)
            build_mask_level(l, hz[:, :, 0:Rp // 2], Rp // 2, Rp // 2)

    # =======================================================================
    # L0 streaming: fill resident xfull = x*m (fp16) superchunk by
    # superchunk; mask zx-pool + counts + channel-major mask dup on the fly.
    # =======================================================================
    w0slc = load_weights(0)     # first on the Act queue: L0 blocks on it
    wslcs = {1: load_weights(1)}

    # constants/g/b after the weight blobs (not needed until post_stats)
    gvec = {}
    for i in range(6):
        C = CH[i + 1]
        gvec[i] = persist.tile([C, 2], F32, name=f"gb{i}", tag=f"gb{i}")
        nc.scalar.dma_start(out=gvec[i][:, 0:1], in_=AP(gb_d[f"g{i}"], 0, [[1, C], [1, 1]]))
        nc.scalar.dma_start(out=gvec[i][:, 1:2], in_=AP(gb_d[f"b{i}"], 0, [[1, C], [1, 1]]))
    for i in range(6):
        cm(f"chmap{i}")
        cm(f"bcast{i}")
    for l in range(1, 6):
        cm(f"pair_u{l}", f16=True)
        for t in range(len(out_tiles_for(l))):
            cm(f"pair_d{l}_{t}", f16=True)
    cm("ones_1_64")
    cm("ones_128_1")

    nc.vector.memset(stats_sum[...], 0.0)
    nc.vector.memset(stats_sq[...], 0.0)

    msups = {}

    def stream_section(sc):
        z_lo = sc * SUP0
        msup = stream.tile([128, SUP0, D], F16, name="m_sup", tag="m_sup")
        dst = xfull[:, 1 + z_lo:1 + z_lo + SUP0, 1:D + 1]
        nc.gpsimd.dma_start(out=dst,
                            in_=AP(x_d, z_lo * D * D, [[D, 128], [D * D, SUP0], [1, D]]))
        nc.gpsimd.dma_start(out=msup[...],
                            in_=AP(m_d, z_lo * D * D, [[D, 128], [D * D, SUP0], [1, D]]))
        # masked input in place (mask is 0/1 so fp16 product is exact)
        nc.vector.tensor_tensor(out=dst, in0=dst, in1=msup[...], op=MULT)
        # mask zx-pool into hp rows z_lo/2 ..
        hx = scratch.tile([128, SUP0, D // 2 + 1], F16, name="hx", tag="hx")
        nc.vector.tensor_tensor(out=hx[:, :, 0:D // 2],
                                in0=msup[:, :, 0:D:2],
                                in1=msup[:, :, 1:D:2], op=MAXOP)
        nc.vector.tensor_tensor(out=hp[:, z_lo // 2:(z_lo + SUP0) // 2, 0:D // 2],
                                in0=hx[:, 0:SUP0:2, 0:D // 2],
                                in1=hx[:, 1:SUP0:2, 0:D // 2], op=MAXOP)
        # active-voxel count partial (per superchunk)
        nc.vector.tensor_reduce(out=cnt_cols[:, sc:sc + 1],
                                in_=msup[...], axis=AX.XY, op=ADD)
        msups[sc] = msup
        # pyramid level 1 incrementally: every 2 superchunks = 8 hp rows
        if sc % 2 == 1:
            mask_level_chunk(1, hp[:, :, 0:D // 2], D // 2,
                             (sc - 1) * SUP0 // 2, (sc + 1) * SUP0 // 2)

    def conv_chunks(sc):
        # conv superchunk sc (reads xfull rows up to 8*sc+9, i.e. needs
        # stream_section(sc+1) issued first for the trailing z-halo)
        for c0 in range(0, SUP0, RCH):
            zc = sc * SUP0 + c0
            chunk_idx = zc // RCH
            for h, (A, B) in enumerate(out_tiles_for(0)):
                ps = psum.tile([128, RCH, D], F32, name="convps", tag="convps")
                mms = []
                for dz, dx in TAPS:
                    rhs = xfull[:, (1 + zc + dz):(1 + zc + dz + RCH),
                                (1 + dx):(1 + dx + D)]
                    outv = ps[...].rearrange("p a b -> p (a b)")
                    mms.append((outv, w0slc[(0, dz, dx, h, 0)], rhs))
                for j, (o, l_, r) in enumerate(mms):
                    mm(o, l_, r, start=(j == 0), stop=(j == len(mms) - 1))
                col = 2 * chunk_idx + h
                # p = (conv + CBIG) * m  -> p0 tile (fp16); accum sum.
                # Two 64-partition ops (one per co) so the mask is read
                # straight from msup (y rows A..B) without a duplicated copy.
                dst = p_tiles[0][h][:, 1 + zc:1 + zc + RCH, 1:D + 1]
                for ch in range(2):
                    pr = slice(ch * 64, (ch + 1) * 64)
                    nc.vector.scalar_tensor_tensor(
                        out=p_tiles[0][h][pr, 1 + zc:1 + zc + RCH, 1:D + 1],
                        in0=ps[pr, :, :], scalar=CBIG,
                        in1=msups[sc][A:B, c0:c0 + RCH, :],
                        op0=ADD, op1=MULT, accum_out=stats_sum[pr, col:col + 1])
                sq = scratch.tile([128, RCH, D], F16, name="sqscr", tag="sqscr")
                nc.scalar.activation(out=sq[...], in_=dst, func=AF.Square,
                                     accum_out=stats_sq[:, col:col + 1])

    for sc in range(D // SUP0):
        stream_section(sc)
        if sc >= 1:
            conv_chunks(sc - 1)
    conv_chunks(D // SUP0 - 1)

    # =======================================================================
    # stats -> AllGather -> s/t vecs  (sums contaminated by CBIG:
    #   S1 = sum(p) = sum(conv*m) + C*cnt ; S2 = sum(p^2))
    # =======================================================================
    def post_stats_issue(i, ncols):
        C = CH[i + 1]
        nc.vector.tensor_reduce(out=stats_red[:, 0:1], in_=stats_sum[:, 0:ncols],
                                axis=AX.X, op=ADD)
        nc.vector.tensor_reduce(out=stats_red[:, 1:2], in_=stats_sq[:, 0:ncols],
                                axis=AX.X, op=ADD)
        pc = ps_small()
        mm(pc[0:C, 0:2], cm(f"chmap{i}")[...], stats_red[...], start=True, stop=True)
        stage = scratch.tile([C, 2], F32, name="ccstage", tag="ccstage")
        nc.vector.tensor_copy(out=stage[...], in_=pc[0:C, 0:2])
        if i == 0:
            # L0 count from the streaming partials, channel-replicated
            r0 = scratch.tile([128, 1], F32, name="cnt0red", tag="cnt0red")
            nc.vector.tensor_reduce(out=r0[...], in_=cnt_cols[...], axis=AX.X, op=ADD)
            po0 = ps_small()
            mm(po0[0:C, 0:1], cm("ones_128_64")[:, 0:C], r0[...], start=True, stop=True)
            nc.vector.tensor_copy(out=cnt_pc[0:C, 0:1], in_=po0[0:C, 0:1])
        # payload layout: per-channel (sum, sumsq, cnt) triplets
        nc.sync.dma_start(out=AP(cc_in[i], 0, [[3, C], [1, 2]]), in_=stage[...])
        nc.sync.dma_start(out=AP(cc_in[i], 2, [[3, C], [1, 1]]),
                          in_=cnt_pc[0:C, i:i + 1])
        L = 3 * C
        if FAKE_AG:
            # timing A/B only (numerically wrong): local broadcast
            nc.sync.dma_start(
                out=AP(cc_out[i], 0, [[L, N_CORES], [1, L]]),
                in_=AP(cc_in[i], 0, [[0, N_CORES], [1, L]]))
        else:
            nc.gpsimd.collective_compute(
                "AllGather", BYPASS, replica_groups=RG,
                ins=[AP(cc_in[i], 0, [[1, 1], [1, L]])],
                outs=[AP(cc_out[i], 0, [[1, 1], [1, N_CORES * L]])])

    def post_stats_finish(i):
        C = CH[i + 1]
        L = 3 * C
        # gathered [8, L] -> g8 [C, 3, 8] (partition=c)
        g8 = scratch.tile([C, 3, 8], F32, name="g8", tag="g8")
        nc.sync.dma_start(out=g8[...],
                          in_=AP(cc_out[i], 0, [[3, C], [1, 3], [L, 8]]))
        st = scratch.tile([C, 3], F32, name="post_st", tag="post_st")
        nc.vector.tensor_reduce(out=st[...], in_=g8[...], axis=AX.X, op=ADD)
        # de-contaminate: s1 = S1 - C*cnt ; s2 = S2 - 2C*s1 - C^2*cnt
        s1 = scratch.tile([C, 1], F32, name="s1v", tag="s1v")
        nc.vector.scalar_tensor_tensor(out=s1[...], in0=st[:, 2:3], scalar=-CBIG,
                                       in1=st[:, 0:1], op0=MULT, op1=ADD)
        s2 = scratch.tile([C, 1], F32, name="s2v", tag="s2v")
        nc.vector.scalar_tensor_tensor(out=s2[...], in0=s1[...], scalar=-2.0 * CBIG,
                                       in1=st[:, 1:2], op0=MULT, op1=ADD)
        nc.vector.scalar_tensor_tensor(out=s2[...], in0=st[:, 2:3], scalar=-CBIG * CBIG,
                                       in1=s2[...], op0=MULT, op1=ADD)
        ccl = scratch.tile([C, 1], F32, name="ccl", tag="ccl")
        nc.vector.tensor_scalar_max(out=ccl[...], in0=st[:, 2:3], scalar1=1.0)
        inv = scratch.tile([C, 1], F32, name="crep", tag="crep")
        nc.vector.reciprocal(out=inv[...], in_=ccl[...])
        mean = scratch.tile([C, 1], F32, name="meanv", tag="meanv")
        nc.vector.tensor_tensor(out=mean[...], in0=s1[...], in1=inv[...], op=MULT)
        var = scratch.tile([C, 1], F32, name="varv", tag="varv")
        nc.vector.tensor_tensor(out=var[...], in0=s2[...], in1=inv[...], op=MULT)
        m2 = scratch.tile([C, 1], F32, name="m2v", tag="m2v")
        nc.vector.tensor_tensor(out=m2[...], in0=mean[...], in1=mean[...], op=MULT)
        nc.vector.tensor_tensor(out=var[...], in0=var[...], in1=m2[...], op=SUB)
        rs = scratch.tile([C, 1], F32, name="rsv", tag="rsv")
        nc.scalar.activation(out=rs[...], in_=var[...], func=AF.Rsqrt,
                             bias=EPS, scale=1.0)
        sv = scratch.tile([C, 2], F32, name="sv2", tag="sv2")
        nc.vector.tensor_tensor(out=sv[:, 0:1], in0=gvec[i][:, 0:1], in1=rs[...], op=MULT)
        ms_ = scratch.tile([C, 1], F32, name="msv", tag="msv")
        nc.vector.tensor_tensor(out=ms_[...], in0=mean[...], in1=sv[:, 0:1], op=MULT)
        nc.vector.tensor_tensor(out=sv[:, 1:2], in0=gvec[i][:, 1:2], in1=ms_[...], op=SUB)
        nc.vector.scalar_tensor_tensor(out=sv[:, 1:2], in0=sv[:, 0:1], scalar=-CBIG,
                                       in1=sv[:, 1:2], op0=MULT, op1=ADD)
        bps = ps_small()
        mm(bps[0:128, 0:2], cm(f"bcast{i}")[...], sv[...], start=True, stop=True)
        nc.vector.tensor_copy(out=stvec[i][...], in_=bps[0:128, 0:2])

    def apply_layer(i, zchunks=1, split_dve=False):
        # in-place BN+ReLU on p_tiles[i] (fp16); pads/inactive stay zero.
        # z-major chunk order so the next conv can start on early z of all
        # tiles as soon as possible. split_dve alternates chunks between the
        # Act and DVE engines to halve the wall time of big applies.
        Zp = SZ[i + 1] + 2
        step = max(1, Zp // zchunks)
        ci = 0
        for z0 in range(0, Zp, step):
            z1 = min(Zp, z0 + step)
            for t, pt in enumerate(p_tiles[i]):
                lo, hi = out_tiles_for(i)[t]
                P = (hi - lo) * CH[i + 1]
                v = pt[0:P, z0:z1, :].rearrange("p a b -> p (a b)")
                if split_dve and ci % 2 == 1:
                    nc.vector.tensor_scalar(out=v, in0=v, scalar1=svec[i][0:P, :],
                                            scalar2=tvec[i][0:P, :], op0=MULT, op1=ADD)
                    nc.vector.tensor_scalar_max(out=v, in0=v, scalar1=0.0)
                else:
                    nc.scalar.activation(out=v, in_=v, func=AF.Relu,
                                         bias=tvec[i][0:P, :], scale=svec[i][0:P, :])
                ci += 1

    # boundary-row staging for layers in BND_LAYERS: the 3 dz-shifted,
    # z-subsampled copies of input row yi = s*A-1 (from prev tile 0) as
    # partitions p = dzi*Ci + ci; x stays full-resolution (contiguous rows
    # so the DMA is 3-dim legal) and the dx shift happens at matmul time.
    # Reuses xfull's buffer (same tag, dead after L0).
    def build_stage(i):
        Ci = CH[i]
        Zo = Xo = SZ[i + 1]
        Xi = SZ[i]
        tlo, thi = in_tiles_for(i)[0]
        per = thi - tlo
        stf = persist.tile([128, D + 2, D + 2], F16, name="stage", tag="xfull")
        src = p_tiles[i - 1][0]
        for dzi, dz in enumerate((-1, 0, 1)):
            zsl = slice(1 + dz, 1 + dz + 2 * (Zo - 1) + 1, 2)
            nc.sync.dma_start(
                out=stf[dzi * Ci:(dzi + 1) * Ci, 0:Zo, 0:Xi + 2],
                in_=src[per - 1:per * Ci:per, zsl, 0:Xi + 2])
        return stf

    # =======================================================================
    # layers 1..5: generic resident conv (input = p_tiles[i-1], relu applied)
    # =======================================================================
    def conv_layer(i, wslc, stage=None):
        Ci, Co, s = CH[i], CH[i + 1], STRIDES[i]
        Zo = Xo = SZ[i + 1]
        nc.vector.memset(stats_sum[...], 0.0)
        nc.vector.memset(stats_sq[...], 0.0)
        RZ = max(1, min(Zo, 512 // Xo))
        col = 0
        for mci, (A, B) in enumerate(out_tiles_for(i)):
            M = (B - A) * Co
            for z0 in range(0, Zo, RZ):
                R = min(RZ, Zo - z0)
                ps = psum.tile([128, RZ, Xo], F32, name="convps2", tag="convps")
                outv = ps[0:M, 0:R, :].rearrange("p a b -> p (a b)")
                mms = []
                for dz, dx in TAPS:
                    zs_ = slice(1 + s * z0 + dz, 1 + s * (z0 + R - 1) + dz + 1, s)
                    xs_ = slice(1 + dx, 1 + dx + s * (Xo - 1) + 1, s)
                    for it in k_tiles_main(i, mci):
                        mms.append((outv, wslc[(i, dz, dx, mci, it)],
                                    p_tiles[i - 1][it][:, zs_, xs_]))
                if stage is not None and mci == 1:
                    K = 3 * Ci
                    for dx in (-1, 0, 1):
                        xs_ = slice(1 + dx, 1 + dx + s * (Xo - 1) + 1, s)
                        mms.append((outv, wslc[(i, 'b', dx, 1, 0)][0:K, :],
                                    stage[0:K, z0:z0 + R, xs_]))
                for j, (o, l_, r) in enumerate(mms):
                    mm(o, l_, r, start=(j == 0), stop=(j == len(mms) - 1))
                # epilogue: p = (conv + CBIG) * m -> p_tiles[i]; stat accums
                dst = p_tiles[i][mci][0:M, 1 + z0:1 + z0 + R, 1:Xo + 1]
                nc.vector.scalar_tensor_tensor(
                    out=dst, in0=ps[0:M, 0:R, :], scalar=CBIG,
                    in1=m_dup[i][mci][0:M, z0:z0 + R, :],
                    op0=ADD, op1=MULT, accum_out=stats_sum[0:M, col:col + 1])
                sq = scratch.tile([128, RZ, Xo], F16, name="sqscr2", tag="sqscrg")
                nc.scalar.activation(out=sq[0:M, 0:R, :], in_=dst, func=AF.Square,
                                     accum_out=stats_sq[0:M, col:col + 1])
                col += 1
        return col

    post_stats_issue(0, 64)
    wslcs[2] = load_weights(2)   # transfers inside the AG(0) window
    build_pyramid_upper()    # overlaps the AllGather of layer-0 stats
    post_stats_finish(0)
    apply_layer(0, zchunks=4, split_dve=True)
    for i in range(1, 6):
        stage = build_stage(i) if i in BND_LAYERS else None
        ncols = conv_layer(i, wslcs[i], stage)
        post_stats_issue(i, ncols)
        if i + 2 <= 5:
            # next-next blob: emitted here so it never delays a stats chain,
            # and its rotation buffer is already free
            wslcs[i + 2] = load_weights(i + 2)
        post_stats_finish(i)
        apply_layer(i, zchunks=4 if i == 1 else 1)

    # =======================================================================
    # final masked maxpool -> out [64(co), 8(yo,zo,xo)]
    # =======================================================================
    y5t = p_tiles[5]   # tiles [128=(co*2+yo), 6, 6] z/x-padded (data 1..4)
    wt2 = scratch.tile([64, 4, 4], F32, name="wt2", tag="wt2")
    for t in range(2):
        src = y5t[t]
        u = scratch.tile([128, 2, 6], F32, name=f"poolu{t}", tag=f"poolu{t}")
        nc.vector.tensor_tensor(out=u[:, 0:1, :], in0=src[:, 1:2, :], in1=src[:, 2:3, :], op=MAXOP)
        nc.vector.tensor_tensor(out=u[:, 1:2, :], in0=src[:, 2:3, :], in1=src[:, 3:4, :], op=MAXOP)
        nc.vector.tensor_tensor(out=u[:, 1:2, :], in0=u[:, 1:2, :], in1=src[:, 4:5, :], op=MAXOP)
        w = scratch.tile([128, 2, 2], F32, name=f"poolw{t}", tag=f"poolw{t}")
        nc.vector.tensor_tensor(out=w[:, :, 0:1], in0=u[:, :, 1:2], in1=u[:, :, 2:3], op=MAXOP)
        nc.vector.tensor_tensor(out=w[:, :, 1:2], in0=u[:, :, 2:3], in1=u[:, :, 3:4], op=MAXOP)
        nc.vector.tensor_tensor(out=w[:, :, 1:2], in0=w[:, :, 1:2], in1=u[:, :, 4:5], op=MAXOP)
        # SBUF->SBUF realign: partitions (co*2+yl) -> wt2[co, 2t+yl, :]
        nc.sync.dma_start(out=wt2[:, 2 * t:2 * t + 2, :],
                          in_=w[...].rearrange("p a b -> p (a b)"))
    fin = scratch.tile([64, 2, 4], F32, name="fin", tag="fin")
    nc.vector.tensor_tensor(out=fin[:, 0:1, :], in0=wt2[:, 0:1, :], in1=wt2[:, 1:2, :], op=MAXOP)
    nc.vector.tensor_tensor(out=fin[:, 1:2, :], in0=wt2[:, 1:2, :], in1=wt2[:, 2:3, :], op=MAXOP)
    nc.vector.tensor_tensor(out=fin[:, 1:2, :], in0=fin[:, 1:2, :], in1=wt2[:, 3:4, :], op=MAXOP)
    m5u = m_undup[5]   # [4, 4, 4] f32 (partitions = yo)
    a1 = scratch.tile([4, 4, 3], F32, name="m5a", tag="m5a")
    nc.vector.tensor_tensor(out=a1[:, :, 0:2], in0=m5u[:, :, 0:4:2], in1=m5u[:, :, 1:4:2], op=MAXOP)
    a2 = scratch.tile([4, 2, 3], F32, name="m5b", tag="m5b")
    nc.vector.tensor_tensor(out=a2[:, :, 0:2], in0=a1[:, 0:4:2, 0:2], in1=a1[:, 1:4:2, 0:2], op=MAXOP)
    m5r = scratch.tile([1, 4, 4], F32, name="m5r", tag="m5r")
    nc.sync.dma_start(out=m5r[...], in_=a2[:, :, 0:2])
    mo = scratch.tile([1, 2, 4], F32, name="mo", tag="mo")
    nc.vector.tensor_tensor(out=mo[:, 0:1, :], in0=m5r[:, 0:1, :], in1=m5r[:, 1:2, :], op=MAXOP)
    nc.vector.tensor_tensor(out=mo[:, 1:2, :], in0=m5r[:, 1:2, :], in1=m5r[:, 2:3, :], op=MAXOP)
    nc.vector.tensor_tensor(out=mo[:, 1:2, :], in0=mo[:, 1:2, :], in1=m5r[:, 3:4, :], op=MAXOP)
    mo_ps = ps_small()
    mm(mo_ps[0:64, 0:8], cm("ones_1_64")[...], mo[...].rearrange("p a b -> p (a b)"),
       start=True, stop=True)
    outt = scratch.tile([64, 8], F32, name="outt", tag="outt")
    nc.vector.tensor_tensor(out=outt[...], in0=fin[...].rearrange("p a b -> p (a b)"),
                            in1=mo_ps[0:64, 0:8], op=MULT)
    nc.sync.dma_start(out=out_d[:, :], in_=outt[...])
    es.close()


# ---------------------------------------------------------------------------
def build_in_maps(inputs, wmeta, c16_meta, c32_meta):
    ws = [np.asarray(inputs[f"w{i}"], np.float32) for i in range(6)]
    wlen = max(o + 128 * n for o, n in wmeta.values())
    wblob = np.zeros(wlen, np.float16)
    for i in range(6):
        off, ncols = wmeta[i]
        wblob[off:off + 128 * ncols] = layer_blob(ws, i).ravel()
    c16 = build_consts16()
    c16len = max(o + int(np.prod(sh)) for o, sh in c16_meta.values())
    c16b = np.zeros(c16len, np.float16)
    for k, (o, sh) in c16_meta.items():
        c16b[o:o + int(np.prod(sh))] = c16[k].astype(np.float16).ravel()
    c32 = build_consts32()
    c32len = max(o + int(np.prod(sh)) for o, sh in c32_meta.values())
    c32b = np.zeros(c32len, np.float32)
    for k, (o, sh) in c32_meta.items():
        c32b[o:o + int(np.prod(sh))] = c32[k].ravel()

    xf = np.ascontiguousarray(np.asarray(inputs["x_feats"], np.float32)[..., 0])
    xm = np.ascontiguousarray(np.asarray(inputs["x_mask"], np.int32)[..., 0])
    in_maps = []
    for c in range(N_CORES):
        im = {"x": xf[c], "m": xm[c], "wblob": wblob, "c16": c16b, "c32": c32b}
        for i in range(6):
            im[f"g{i}"] = np.asarray(inputs[f"g{i}"], np.float32)
            im[f"b{i}"] = np.asarray(inputs[f"b{i}"], np.float32)
        in_maps.append(im)
    return in_maps


def unpack_out(res):
    outs = []
    for c in range(N_CORES):
        o = res[c].reshape(64, 2, 2, 2)        # [co, yo, zo, xo]
        outs.append(np.transpose(o, (2, 1, 3, 0)))  # [zo, yo, xo, co]
    return np.stack(outs).astype(np.float32)


def kernel(**inputs):
    if "prog" not in _CACHE:
        _CACHE["prog"] = build_program()
    nc, wmeta, c16_meta, c32_meta = _CACHE["prog"]
    in_maps = build_in_maps(inputs, wmeta, c16_meta, c32_meta)
    globals()["_last_in_maps"] = in_maps
    res = run_bass_kernel_spmd(nc, in_maps, list(range(N_CORES)))
    return unpack_out([res.results[c]["out"] for c in range(N_CORES)])
